# revision 19
# baseline (speedup 1.0000x reference)
"""CenterNet-style NMS detection head on 8 Trainium2 NeuronCores.

Per sample (pure data parallel, 1 sample per core):
  1. argmax over heatmap[:, 1] (512x512).  The reference's 3x3 max-pool NMS
     is provably a no-op for the subsequent argmax: the global max is always
     its own local max and every tied-at-max position survives the pool, so
     argmax(nms(heat)) == argmax(heat) including first-index tie-breaking.
  2. top/left = clip(c - 64, 0, 383); crop feature_map[:, top:top+128,
     left:left+128] via register-offset (dynamic) DMA.
  3. disc mask (r = clip(floor(wh[0,cx,cy] + 8), 0, 254)) and multiply.
     When r <= 15 the non-zero region fits a 32-row window of the crop, so
     only those rows are read/written (the output buffer is zero-filled by
     the runtime); otherwise a full 128-row fallback path runs.
  4. hook_coord = [top, top+128, left, left+128].

Structural notes:
  - heat is DMAed in 4 chunks, each chunk's per-partition top-8 computed as
    it lands, so the reduction overlaps the load.
  - the r_val gather is issued as soon as cx/cy are known; the rad-
    independent part of the disc mask is computed while it is in flight.
  - the fast path packs the 32-row window as 4 channel-groups across all
    128 partitions so the masked multiply runs at full DVE width.
  - the If arms keep identical DMA/engine instruction counts: Tile merges
    the arms' semaphore clocks, so asymmetric arms would leave the epilogue
    waiting on ticks the taken arm never produces.
  - "bridge" copies absorb DMA-completion waits so heavier compute
    instructions carry a single wait (hardware limit; Bacc would otherwise
    legalize via slower standalone EVENT_SEMAPHORE chains).
"""

import sys

sys.path.insert(0, "/opt/trn_rl_repo")

import numpy as np

import concourse.bacc as bacc
import concourse.bass as bass
import concourse.mybir as mybir
from concourse.bass_types import DynSlice
from concourse.tile import TileContext
from concourse.tile_rust import add_dep_helper

f32 = mybir.dt.float32
i32 = mybir.dt.int32
u32 = mybir.dt.uint32
Alu = mybir.AluOpType

B, C, H, W = 8, 32, 512, 512
RADIUS = 64
CROP = 2 * RADIUS  # 128
R_EXPAND = 8
R_MAX = H // 2 - 1  # 255
TOPMAX = H - 1 - 2 * RADIUS  # 383
WIN = 32  # fast-path row window (covers disc when rad <= 15)
CG = C // 4  # channels per fast-path partition group (8)

N_CORES = 8

ds = DynSlice

# engines that need the rad register: every engine with instructions inside
# the If bodies (tc.If branches exactly on the engines holding the register)
RADV_ENGINES = [mybir.EngineType.SP, mybir.EngineType.DVE]


def _consts_np() -> np.ndarray:
    cst = np.zeros((128, 257), np.float32)
    cst[:, 0] = np.arange(128)  # iota over partitions
    cst[:, 1:129] = np.arange(128)[None, :]  # iota over free dim (per row)
    cst[:, 129:257] = np.eye(128)  # identity for PE transpose
    return cst


def build_nc(use_if: bool = True) -> bass.Bass:
    # Bacc (not raw Bass): its compile() legalizes sync waits (TRN2 allows
    # at most one wait per instruction; extra waits become standalone
    # EVENT_SEMAPHORE instructions) and runs register allocation.
    nc = bacc.Bacc()
    heat = nc.declare_dram_parameter("heat", [H, W], f32, isOutput=False)
    wh = nc.declare_dram_parameter("wh", [H, W], f32, isOutput=False)
    fm = nc.declare_dram_parameter("fm", [C, H, W], f32, isOutput=False)
    consts = nc.declare_dram_parameter("consts", [128, 257], f32, isOutput=False)
    out = nc.declare_dram_parameter("out", [C, CROP, CROP], f32, isOutput=True)
    hook = nc.declare_dram_parameter("hook", [4], i32, isOutput=True)

    SP = [mybir.EngineType.SP]

    with TileContext(nc) as tc:
        with (
            tc.tile_pool(name="sb", bufs=1) as pool,
            tc.tile_pool(name="ps", bufs=1, space="PSUM") as psum,
        ):
            # ---------- constants ----------
            cst = pool.tile([128, 257], f32)
            nc.sync.dma_start(cst, consts[:, :])
            iota_p = cst[:, 0:1]
            iota_f = cst[:, 1:129]
            ident = cst[:, 129:257]
            ones = pool.tile([1, 128], f32)
            nc.vector.memset(ones, 1.0)

            # ---------- heat: [512, 512] -> [128 partitions, 2048] ----------
            # partition p holds rows 4p..4p+3, so free order == row-major
            # (flat) order within a partition and flat = 2048*p + f.
            # 4 chunked loads; each chunk's top-8 runs while the next loads.
            heat_r = heat.rearrange("(p a) w -> p (a w)", p=128)
            heat_sb = pool.tile([128, 2048], f32)
            mx4 = pool.tile([128, 32], f32)
            for c in range(4):
                sl = slice(512 * c, 512 * (c + 1))
                nc.sync.dma_start(heat_sb[:, sl], heat_r[:, sl])
                nc.vector.max(out=mx4[:, 8 * c : 8 * c + 8], in_=heat_sb[:, sl])

            # per-partition top-1 value + FIRST index within the partition
            maxv = pool.tile([128, 8], f32)
            maxi = pool.tile([128, 8], u32)
            nc.vector.max(out=maxv, in_=mx4[:, :])
            nc.vector.max_index(maxi, maxv, heat_sb[:, :])

            idxf = pool.tile([128, 1], f32)
            nc.vector.tensor_copy(idxf, maxi[:, 0:1])  # u32 -> f32 (exact, <2048)
            flatf = pool.tile([128, 1], f32)
            nc.vector.tensor_scalar(flatf, iota_p, 2048.0, None, op0=Alu.mult)
            nc.vector.tensor_add(flatf, flatf, idxf)  # flat, exact in f32 (<2^18)

            # ---------- cross-partition argmax (value desc, flat asc) ----------
            # single-column PE transposes (engine APs must start at quadrant-
            # aligned partitions, so a [2,128] PSUM tile is unusable).
            # Warm-up transpose makes PE observe the const DMA once so each
            # later matmul carries a single cross-engine wait.
            tpw = psum.tile([1, 128], f32)
            nc.tensor.transpose(tpw, iota_p, ident)
            tpv = psum.tile([1, 128], f32)
            tpf = psum.tile([1, 128], f32)
            nc.tensor.transpose(tpv, maxv[:, 0:1], ident)
            nc.tensor.transpose(tpf, flatf, ident)
            valrow = pool.tile([1, 128], f32)
            flatrow = pool.tile([1, 128], f32)
            nc.vector.tensor_copy(valrow, tpv[0:1, :])
            nc.vector.tensor_copy(flatrow, tpf[0:1, :])

            gm8 = pool.tile([1, 8], f32)
            nc.vector.max(out=gm8, in_=valrow[:, :])
            ismax = pool.tile([1, 128], u32)  # copy_predicated wants int mask
            nc.vector.tensor_scalar(ismax, valrow, gm8[0:1, 0:1], None, op0=Alu.is_ge)
            candrow = pool.tile([1, 128], f32)
            nc.vector.memset(candrow, 1.0e9)
            nc.vector.copy_predicated(candrow, ismax, flatrow)
            negrow = pool.tile([1, 128], f32)
            nc.vector.tensor_scalar(negrow, candrow, -1.0, None, op0=Alu.mult)
            mn8 = pool.tile([1, 8], f32)
            nc.vector.max(out=mn8, in_=negrow[:, :])

            # ---------- scalar pipeline (all [1,1] on partition 0) ----------
            Sf = pool.tile([1, 8], f32)  # f32 scratch
            Si = pool.tile([1, 16], i32)  # i32 scratch
            # Si columns: 0 flat, 1 cx, 2 cy, 3 top, 4 left, 5 center_r,
            # 6 center_c, 7 wr, 8 gr0, 9 c1, 10 isgt, 11 rad, 12 dRw, 13 dRs,
            # 14 dC
            nc.vector.tensor_scalar(Sf[:, 0:1], mn8[0:1, 0:1], -1.0, None, op0=Alu.mult)
            nc.vector.tensor_copy(Si[:, 0:1], Sf[:, 0:1])  # f32 -> i32 exact
            nc.vector.tensor_scalar(
                Si[:, 1:2], Si[:, 0:1], 9, None, op0=Alu.logical_shift_right
            )  # cx = flat >> 9
            nc.vector.tensor_scalar(
                Si[:, 2:3], Si[:, 0:1], 511, None, op0=Alu.bitwise_and
            )  # cy = flat & 511

            # r_val gather launches as early as possible; everything below
            # that does not need rad overlaps its flight.
            cxv = nc.values_load(
                Si[:, 1:2],
                engines=SP,
                min_val=0,
                max_val=H - 1,
                skip_runtime_bounds_check=True,
            )
            cyv = nc.values_load(
                Si[:, 2:3],
                engines=SP,
                min_val=0,
                max_val=W - 1,
                skip_runtime_bounds_check=True,
            )
            rv = pool.tile([1, 1], f32)
            nc.sync.dma_start(rv, wh[ds(cxv, 1), ds(cyv, 1)])

            # top = clip(cx - 64, 0, 383); left = clip(cy - 64, 0, 383)
            nc.vector.tensor_scalar(
                Si[:, 3:4], Si[:, 1:2], RADIUS, 0, op0=Alu.subtract, op1=Alu.max
            )
            nc.vector.tensor_scalar(Si[:, 3:4], Si[:, 3:4], TOPMAX, None, op0=Alu.min)
            nc.vector.tensor_scalar(
                Si[:, 4:5], Si[:, 2:3], RADIUS, 0, op0=Alu.subtract, op1=Alu.max
            )
            nc.vector.tensor_scalar(Si[:, 4:5], Si[:, 4:5], TOPMAX, None, op0=Alu.min)
            # centers in crop coords
            nc.vector.tensor_tensor(Si[:, 5:6], Si[:, 1:2], Si[:, 3:4], op=Alu.subtract)
            nc.vector.tensor_tensor(Si[:, 6:7], Si[:, 2:3], Si[:, 4:5], op=Alu.subtract)
            # wr = clip(center_r - 16, 0, 96); gr0 = top + wr
            nc.vector.tensor_scalar(
                Si[:, 7:8], Si[:, 5:6], 16, 0, op0=Alu.subtract, op1=Alu.max
            )
            nc.vector.tensor_scalar(Si[:, 7:8], Si[:, 7:8], CROP - WIN, None, op0=Alu.min)
            nc.vector.tensor_tensor(Si[:, 8:9], Si[:, 3:4], Si[:, 7:8], op=Alu.add)

            # ---------- hook coords ----------
            hook_t = pool.tile([1, 4], i32)
            nc.vector.tensor_copy(hook_t[:, 0:1], Si[:, 3:4])
            nc.vector.tensor_scalar(hook_t[:, 1:2], Si[:, 3:4], CROP, None, op0=Alu.add)
            nc.vector.tensor_copy(hook_t[:, 2:3], Si[:, 4:5])
            nc.vector.tensor_scalar(hook_t[:, 3:4], Si[:, 4:5], CROP, None, op0=Alu.add)
            nc.sync.dma_start(hook[:], hook_t)

            # crop registers (available well before rad)
            topv = nc.values_load(
                Si[:, 3:4],
                engines=SP,
                min_val=0,
                max_val=TOPMAX,
                skip_runtime_bounds_check=True,
            )
            leftv = nc.values_load(
                Si[:, 4:5],
                engines=SP,
                min_val=0,
                max_val=TOPMAX,
                skip_runtime_bounds_check=True,
            )
            grv = nc.values_load(
                Si[:, 8:9],
                engines=SP,
                min_val=0,
                max_val=TOPMAX + CROP - WIN,
                skip_runtime_bounds_check=True,
            )
            wrv = nc.values_load(
                Si[:, 7:8],
                engines=SP,
                min_val=0,
                max_val=CROP - WIN,
                skip_runtime_bounds_check=True,
            )

            # mask base deltas: fast dR = gr0 - cx, slow dR = top - cx,
            # shared dC = left - cy
            nc.vector.tensor_tensor(Si[:, 12:13], Si[:, 8:9], Si[:, 1:2], op=Alu.subtract)
            nc.vector.tensor_tensor(Si[:, 13:14], Si[:, 3:4], Si[:, 1:2], op=Alu.subtract)
            nc.vector.tensor_tensor(Si[:, 14:15], Si[:, 4:5], Si[:, 2:3], op=Alu.subtract)
            scal2f = pool.tile([1, 2], f32)
            scal2s = pool.tile([1, 2], f32)
            nc.vector.tensor_copy(scal2f[:, 0:1], Si[:, 12:13])
            nc.vector.tensor_copy(scal2f[:, 1:2], Si[:, 14:15])
            nc.vector.tensor_copy(scal2s[:, 0:1], Si[:, 13:14])
            nc.vector.tensor_copy(scal2s[:, 1:2], Si[:, 14:15])

            # broadcast deltas across partitions via K=1 matmuls; PSUM is
            # staged through SBUF so mask math carries DVE-only waits.
            # bc_sb columns: 0 dRf, 1 dC, 2 dRs, 3 dC, 4 r2
            bcf = psum.tile([128, 2], f32)
            bcs = psum.tile([128, 2], f32)
            nc.tensor.matmul(bcf, ones, scal2f, start=True, stop=True)
            nc.tensor.matmul(bcs, ones, scal2s, start=True, stop=True)
            bc_sb = pool.tile([128, 5], f32)
            nc.vector.tensor_copy(bc_sb[:, 0:2], bcf[:, :])
            nc.vector.tensor_copy(bc_sb[:, 2:4], bcs[:, :])

            # rad-independent mask halves: sum2 = (dR+p)^2 + (dC+c)^2
            dr = pool.tile([128, 1], f32)
            dr2 = pool.tile([128, 1], f32)
            dc = pool.tile([128, CROP], f32)
            sum2F = pool.tile([128, CROP], f32)
            sum2S = pool.tile([128, CROP], f32)

            def emit_sum2(drcol, dccol, sum2):
                nc.vector.tensor_tensor(dr, iota_p, drcol, op=Alu.add)
                nc.vector.tensor_tensor(dr2, dr, dr, op=Alu.mult)
                nc.vector.tensor_scalar(dc, iota_f, dccol, None, op0=Alu.add)
                nc.vector.tensor_tensor(sum2, dc, dc, op=Alu.mult)
                nc.vector.tensor_tensor(
                    sum2, sum2, dr2.to_broadcast([128, CROP]), op=Alu.add
                )

            emit_sum2(bc_sb[:, 0:1], bc_sb[:, 1:2], sum2F)
            emit_sum2(bc_sb[:, 2:3], bc_sb[:, 3:4], sum2S)

            # ---------- rad = clip(floor(r_val + 8), 0, 254) ----------
            nc.vector.tensor_scalar(Sf[:, 1:2], rv, float(R_EXPAND), None, op0=Alu.add)
            nc.vector.tensor_copy(Si[:, 9:10], Sf[:, 1:2])  # c1 (unknown rounding)
            nc.vector.tensor_copy(Sf[:, 2:3], Si[:, 9:10])  # c1 back to f32
            nc.vector.tensor_tensor(
                Sf[:, 3:4], Sf[:, 2:3], Sf[:, 1:2], op=Alu.is_gt
            )  # c1 > x -> overshoot by 1
            nc.vector.tensor_copy(Si[:, 10:11], Sf[:, 3:4])
            nc.vector.tensor_tensor(
                Si[:, 11:12], Si[:, 9:10], Si[:, 10:11], op=Alu.subtract
            )  # floor
            nc.vector.tensor_scalar(
                Si[:, 11:12], Si[:, 11:12], 0, R_MAX - 1, op0=Alu.max, op1=Alu.min
            )
            nc.vector.tensor_copy(Sf[:, 4:5], Si[:, 11:12])  # rad as f32
            r2sb = pool.tile([1, 1], f32)
            nc.vector.tensor_tensor(r2sb, Sf[:, 4:5], Sf[:, 4:5], op=Alu.mult)
            bc2 = psum.tile([128, 1], f32)
            nc.tensor.matmul(bc2, ones, r2sb, start=True, stop=True)
            nc.vector.tensor_copy(bc_sb[:, 4:5], bc2[:, :])

            radv = nc.values_load(
                Si[:, 11:12],
                engines=RADV_ENGINES,
                min_val=0,
                max_val=R_MAX - 1,
                skip_runtime_bounds_check=True,
            )

            # final masks (1.0 inside the disc)
            maskF = pool.tile([128, CROP], f32)
            maskS = pool.tile([128, CROP], f32)
            nc.vector.tensor_scalar(maskF, sum2F, bc_sb[:, 4:5], None, op0=Alu.is_le)
            nc.vector.tensor_scalar(maskS, sum2S, bc_sb[:, 4:5], None, op0=Alu.is_le)

            # fast-path mask replicated to all 4 partition groups
            mask4 = pool.tile([128, CROP], f32)
            nc.vector.tensor_copy(mask4[0:32, :], maskF[0:32, :])
            nc.vector.tensor_copy(mask4[32:64, :], maskF[0:32, :])
            nc.vector.tensor_copy(mask4[64:96, :], maskF[0:32, :])
            nc.vector.tensor_copy(mask4[96:128, :], maskF[0:32, :])

            # ---------- per-path tiles (disjoint across the If arms) ----------
            # fast: [128, CG*CROP]; partition 32g+r = window row r, channels
            #       [CG*g, CG*(g+1)); free = (c_lo, col)
            crop_tF = pool.tile([128, CG * CROP], f32)
            out_tF = pool.tile([128, CG * CROP], f32)
            junkF = pool.tile([128, 1], f32)
            # slow: [128, C*CROP]; partition = crop row; free = (chan, col)
            crop_tS = pool.tile([128, C * CROP], f32)
            out_tS = pool.tile([128, C * CROP], f32)
            junkS = pool.tile([128, 1], f32)

            fm_rows = fm.transpose([1, 0, 2])  # [H, C, W]
            out_rows = out.transpose([1, 0, 2])  # [CROP, C, CROP]

            def fast_body():
                # only rows [wr, wr+32) of the crop are non-zero; packed as
                # 4 channel groups x 32 rows across all 128 partitions
                bridges = []
                for g in range(4):
                    ps = slice(32 * g, 32 * (g + 1))
                    nc.sync.dma_start(
                        crop_tF[ps, :],
                        fm_rows[ds(grv, WIN), CG * g : CG * (g + 1), ds(leftv, CROP)],
                    )
                    bridges.append(
                        nc.vector.tensor_copy(
                            junkF[32 * g : 32 * g + 1, :],
                            crop_tF[32 * g : 32 * g + 1, 0:1],
                        )
                    )
                mul = nc.vector.tensor_tensor(
                    out_tF[:, :],
                    crop_tF[:, :],
                    mask4[:, :]
                    .rearrange("p (o w) -> p o w", o=1)
                    .to_broadcast([128, CG, CROP]),
                    op=Alu.mult,
                )
                for br in bridges:
                    add_dep_helper(mul.ins, br.ins, sync=False, reason="wait bridge")
                for g in range(4):
                    ps = slice(32 * g, 32 * (g + 1))
                    nc.sync.dma_start(
                        out_rows[ds(wrv, WIN), CG * g : CG * (g + 1), :],
                        out_tF[ps, :],
                    )

            def slow_body():
                # full 128x128 crop, 4 channel-group DMAs each way (keeps the
                # arm's DMA count equal to the fast arm's)
                bridges = []
                for g in range(4):
                    fs = slice(CG * CROP * g, CG * CROP * (g + 1))
                    nc.sync.dma_start(
                        crop_tS[:, fs],
                        fm_rows[ds(topv, CROP), CG * g : CG * (g + 1), ds(leftv, CROP)],
                    )
                    bridges.append(
                        nc.vector.tensor_copy(
                            junkS[0:1, :], crop_tS[0:1, CG * CROP * g : CG * CROP * g + 1]
                        )
                    )
                mul = nc.vector.tensor_tensor(
                    out_tS[:, :],
                    crop_tS[:, :],
                    maskS[:, :]
                    .rearrange("p (o w) -> p o w", o=1)
                    .to_broadcast([128, C, CROP]),
                    op=Alu.mult,
                )
                for br in bridges:
                    add_dep_helper(mul.ins, br.ins, sync=False, reason="wait bridge")
                for g in range(4):
                    fs = slice(CG * CROP * g, CG * CROP * (g + 1))
                    nc.sync.dma_start(
                        out_rows[:, CG * g : CG * (g + 1), :], out_tS[:, fs]
                    )

            if use_if:
                with tc.If(radv < 16) as cmp:
                    fast_body()
                with cmp.Else():
                    slow_body()
            else:
                slow_body()

    nc.compile()
    return nc


_NC_CACHE = None


def _get_nc() -> bass.Bass:
    global _NC_CACHE
    if _NC_CACHE is None:
        _NC_CACHE = build_nc()
    return _NC_CACHE


def make_in_maps(feature_map, heatmap, outer_wh):
    cst = _consts_np()
    in_maps = []
    for i in range(N_CORES):
        in_maps.append(
            {
                "heat": np.ascontiguousarray(heatmap[i, 1]).astype(np.float32),
                "wh": np.ascontiguousarray(outer_wh[i, 0]).astype(np.float32),
                "fm": np.ascontiguousarray(feature_map[i]).astype(np.float32),
                "consts": cst,
            }
        )
    return in_maps


def run(feature_map, heatmap, outer_wh, trace=False):
    from concourse.bass_utils import run_bass_kernel_spmd

    nc = _get_nc()
    in_maps = make_in_maps(feature_map, heatmap, outer_wh)
    res = run_bass_kernel_spmd(nc, in_maps, list(range(N_CORES)), trace=trace)
    focus = np.stack([np.asarray(res.results[i]["out"]) for i in range(N_CORES)])
    hook = np.stack(
        [np.asarray(res.results[i]["hook"]).astype(np.int32) for i in range(N_CORES)]
    )
    return (focus, hook), res


def kernel(feature_map, heatmap, outer_wh):
    (focus, hook), _ = run(feature_map, heatmap, outer_wh, trace=False)
    return focus, hook


# revision 21
# speedup vs baseline: 1.2784x; 1.2784x over previous
"""CenterNet-style NMS detection head on 8 Trainium2 NeuronCores.

Per sample (pure data parallel, 1 sample per core):
  1. argmax over heatmap[:, 1] (512x512).  The reference's 3x3 max-pool NMS
     is provably a no-op for the subsequent argmax: the global max is always
     its own local max and every tied-at-max position survives the pool, so
     argmax(nms(heat)) == argmax(heat) including first-index tie-breaking.
  2. top/left = clip(c - 64, 0, 383); crop feature_map[:, top:top+128,
     left:left+128] via register-offset (dynamic) DMA.
  3. disc mask (r = clip(floor(wh[0,cx,cy] + 8), 0, 254)) and multiply.
     When r <= 15 the non-zero region fits a 32-row window of the crop, so
     only those rows are read/written (the output buffer is zero-filled by
     the runtime); otherwise a full 128-row fallback path runs.
  4. hook_coord = [top, top+128, left, left+128].

Structural notes:
  - heat is DMAed in 4 chunks, each chunk's per-partition top-8 computed as
    it lands, so the reduction overlaps the load.
  - the r_val gather is issued as soon as cx/cy are known; the rad-
    independent part of the disc mask is computed while it is in flight.
  - the fast path packs the 32-row window as 4 channel-groups across all
    128 partitions so the masked multiply runs at full DVE width.
  - the If arms keep identical DMA/engine instruction counts: Tile merges
    the arms' semaphore clocks, so asymmetric arms would leave the epilogue
    waiting on ticks the taken arm never produces.
  - "bridge" copies absorb DMA-completion waits so heavier compute
    instructions carry a single wait (hardware limit; Bacc would otherwise
    legalize via slower standalone EVENT_SEMAPHORE chains).
"""

import sys

sys.path.insert(0, "/opt/trn_rl_repo")

import numpy as np

import concourse.bacc as bacc
import concourse.bass as bass
import concourse.mybir as mybir
from concourse.bass_types import DynSlice
from concourse.tile import TileContext
from concourse.tile_rust import add_dep_helper

f32 = mybir.dt.float32
i32 = mybir.dt.int32
u32 = mybir.dt.uint32
Alu = mybir.AluOpType

B, C, H, W = 8, 32, 512, 512
RADIUS = 64
CROP = 2 * RADIUS  # 128
R_EXPAND = 8
R_MAX = H // 2 - 1  # 255
TOPMAX = H - 1 - 2 * RADIUS  # 383
WIN = 32  # fast-path row window (covers disc when rad <= 15)
CG = C // 4  # channels per fast-path partition group (8)

N_CORES = 8

ds = DynSlice

# engines that need the rad register: every engine with instructions inside
# the If bodies (tc.If branches exactly on the engines holding the register)
RADV_ENGINES = [
    mybir.EngineType.SP,
    mybir.EngineType.Activation,
    mybir.EngineType.DVE,
]


def _consts_np() -> np.ndarray:
    cst = np.zeros((128, 257), np.float32)
    cst[:, 0] = np.arange(128)  # iota over partitions
    cst[:, 1:129] = np.arange(128)[None, :]  # iota over free dim (per row)
    cst[:, 129:257] = np.eye(128)  # identity for PE transpose
    return cst


def build_nc(use_if: bool = True) -> bass.Bass:
    # Bacc (not raw Bass): its compile() legalizes sync waits (TRN2 allows
    # at most one wait per instruction; extra waits become standalone
    # EVENT_SEMAPHORE instructions) and runs register allocation.
    nc = bacc.Bacc()
    heat = nc.declare_dram_parameter("heat", [H, W], f32, isOutput=False)
    wh = nc.declare_dram_parameter("wh", [H, W], f32, isOutput=False)
    fm = nc.declare_dram_parameter("fm", [C, H, W], f32, isOutput=False)
    consts = nc.declare_dram_parameter("consts", [128, 257], f32, isOutput=False)
    out = nc.declare_dram_parameter("out", [C, CROP, CROP], f32, isOutput=True)
    hook = nc.declare_dram_parameter("hook", [4], i32, isOutput=True)

    SP = [mybir.EngineType.SP]
    SPACT = [mybir.EngineType.SP, mybir.EngineType.Activation]

    with TileContext(nc) as tc:
        with (
            tc.tile_pool(name="sb", bufs=1) as pool,
            tc.tile_pool(name="ps", bufs=1, space="PSUM") as psum,
        ):
            # ---------- constants ----------
            cst = pool.tile([128, 257], f32)
            nc.sync.dma_start(cst, consts[:, :])
            iota_p = cst[:, 0:1]
            iota_f = cst[:, 1:129]
            ident = cst[:, 129:257]
            ones = pool.tile([1, 128], f32)
            nc.vector.memset(ones, 1.0)

            # ---------- heat: [512, 512] -> [128 partitions, 2048] ----------
            # partition p holds rows 4p..4p+3, so free order == row-major
            # (flat) order within a partition and flat = 2048*p + f.
            # 4 chunked loads; each chunk's top-8 runs while the next loads.
            heat_r = heat.rearrange("(p a) w -> p (a w)", p=128)
            heat_sb = pool.tile([128, 2048], f32)
            mx4 = pool.tile([128, 32], f32)
            for c in range(4):
                sl = slice(512 * c, 512 * (c + 1))
                nc.sync.dma_start(heat_sb[:, sl], heat_r[:, sl])
                nc.vector.max(out=mx4[:, 8 * c : 8 * c + 8], in_=heat_sb[:, sl])

            # per-partition top-1 value + FIRST index within the partition
            maxv = pool.tile([128, 8], f32)
            maxi = pool.tile([128, 8], u32)
            nc.vector.max(out=maxv, in_=mx4[:, :])
            nc.vector.max_index(maxi, maxv, heat_sb[:, :])

            idxf = pool.tile([128, 1], f32)
            nc.vector.tensor_copy(idxf, maxi[:, 0:1])  # u32 -> f32 (exact, <2048)
            flatf = pool.tile([128, 1], f32)
            nc.vector.tensor_scalar(flatf, iota_p, 2048.0, None, op0=Alu.mult)
            nc.vector.tensor_add(flatf, flatf, idxf)  # flat, exact in f32 (<2^18)

            # ---------- cross-partition argmax (value desc, flat asc) ----------
            # single-column PE transposes (engine APs must start at quadrant-
            # aligned partitions, so a [2,128] PSUM tile is unusable).
            # Warm-up transpose makes PE observe the const DMA once so each
            # later matmul carries a single cross-engine wait.
            tpw = psum.tile([1, 128], f32)
            nc.tensor.transpose(tpw, iota_p, ident)
            tpv = psum.tile([1, 128], f32)
            tpf = psum.tile([1, 128], f32)
            nc.tensor.transpose(tpv, maxv[:, 0:1], ident)
            nc.tensor.transpose(tpf, flatf, ident)
            valrow = pool.tile([1, 128], f32)
            flatrow = pool.tile([1, 128], f32)
            nc.vector.tensor_copy(valrow, tpv[0:1, :])
            nc.vector.tensor_copy(flatrow, tpf[0:1, :])

            gm8 = pool.tile([1, 8], f32)
            nc.vector.max(out=gm8, in_=valrow[:, :])
            ismax = pool.tile([1, 128], u32)  # copy_predicated wants int mask
            nc.vector.tensor_scalar(ismax, valrow, gm8[0:1, 0:1], None, op0=Alu.is_ge)
            candrow = pool.tile([1, 128], f32)
            nc.vector.memset(candrow, 1.0e9)
            nc.vector.copy_predicated(candrow, ismax, flatrow)
            negrow = pool.tile([1, 128], f32)
            nc.vector.tensor_scalar(negrow, candrow, -1.0, None, op0=Alu.mult)
            mn8 = pool.tile([1, 8], f32)
            nc.vector.max(out=mn8, in_=negrow[:, :])

            # ---------- scalar pipeline (all [1,1] on partition 0) ----------
            Sf = pool.tile([1, 8], f32)  # f32 scratch
            Si = pool.tile([1, 16], i32)  # i32 scratch
            # Si columns: 0 flat, 1 cx, 2 cy, 3 top, 4 left, 5 center_r,
            # 6 center_c, 7 wr, 8 gr0, 9 c1, 10 isgt, 11 rad, 12 dRw, 13 dRs,
            # 14 dC
            nc.vector.tensor_scalar(Sf[:, 0:1], mn8[0:1, 0:1], -1.0, None, op0=Alu.mult)
            nc.vector.tensor_copy(Si[:, 0:1], Sf[:, 0:1])  # f32 -> i32 exact
            nc.vector.tensor_scalar(
                Si[:, 1:2], Si[:, 0:1], 9, None, op0=Alu.logical_shift_right
            )  # cx = flat >> 9
            nc.vector.tensor_scalar(
                Si[:, 2:3], Si[:, 0:1], 511, None, op0=Alu.bitwise_and
            )  # cy = flat & 511

            # r_val gather launches as early as possible; everything below
            # that does not need rad overlaps its flight.
            cxv = nc.values_load(
                Si[:, 1:2],
                engines=SPACT,
                min_val=0,
                max_val=H - 1,
                skip_runtime_bounds_check=True,
            )
            cyv = nc.values_load(
                Si[:, 2:3],
                engines=SPACT,
                min_val=0,
                max_val=W - 1,
                skip_runtime_bounds_check=True,
            )
            rv = pool.tile([1, 1], f32)
            nc.scalar.dma_start(rv, wh[ds(cxv, 1), ds(cyv, 1)])

            # top = clip(cx - 64, 0, 383); left = clip(cy - 64, 0, 383)
            nc.vector.tensor_scalar(
                Si[:, 3:4], Si[:, 1:2], RADIUS, 0, op0=Alu.subtract, op1=Alu.max
            )
            nc.vector.tensor_scalar(Si[:, 3:4], Si[:, 3:4], TOPMAX, None, op0=Alu.min)
            nc.vector.tensor_scalar(
                Si[:, 4:5], Si[:, 2:3], RADIUS, 0, op0=Alu.subtract, op1=Alu.max
            )
            nc.vector.tensor_scalar(Si[:, 4:5], Si[:, 4:5], TOPMAX, None, op0=Alu.min)
            # centers in crop coords
            nc.vector.tensor_tensor(Si[:, 5:6], Si[:, 1:2], Si[:, 3:4], op=Alu.subtract)
            nc.vector.tensor_tensor(Si[:, 6:7], Si[:, 2:3], Si[:, 4:5], op=Alu.subtract)
            # wr = clip(center_r - 16, 0, 96); gr0 = top + wr
            nc.vector.tensor_scalar(
                Si[:, 7:8], Si[:, 5:6], 16, 0, op0=Alu.subtract, op1=Alu.max
            )
            nc.vector.tensor_scalar(Si[:, 7:8], Si[:, 7:8], CROP - WIN, None, op0=Alu.min)
            nc.vector.tensor_tensor(Si[:, 8:9], Si[:, 3:4], Si[:, 7:8], op=Alu.add)

            # ---------- hook coords ----------
            hook_t = pool.tile([1, 4], i32)
            nc.vector.tensor_copy(hook_t[:, 0:1], Si[:, 3:4])
            nc.vector.tensor_scalar(hook_t[:, 1:2], Si[:, 3:4], CROP, None, op0=Alu.add)
            nc.vector.tensor_copy(hook_t[:, 2:3], Si[:, 4:5])
            nc.vector.tensor_scalar(hook_t[:, 3:4], Si[:, 4:5], CROP, None, op0=Alu.add)
            nc.scalar.dma_start(hook[:], hook_t)

            # crop registers (available well before rad)
            topv = nc.values_load(
                Si[:, 3:4],
                engines=SPACT,
                min_val=0,
                max_val=TOPMAX,
                skip_runtime_bounds_check=True,
            )
            leftv = nc.values_load(
                Si[:, 4:5],
                engines=SPACT,
                min_val=0,
                max_val=TOPMAX,
                skip_runtime_bounds_check=True,
            )
            grv = nc.values_load(
                Si[:, 8:9],
                engines=SPACT,
                min_val=0,
                max_val=TOPMAX + CROP - WIN,
                skip_runtime_bounds_check=True,
            )
            wrv = nc.values_load(
                Si[:, 7:8],
                engines=SPACT,
                min_val=0,
                max_val=CROP - WIN,
                skip_runtime_bounds_check=True,
            )

            # mask base deltas: fast dR = gr0 - cx, slow dR = top - cx,
            # shared dC = left - cy
            nc.vector.tensor_tensor(Si[:, 12:13], Si[:, 8:9], Si[:, 1:2], op=Alu.subtract)
            nc.vector.tensor_tensor(Si[:, 13:14], Si[:, 3:4], Si[:, 1:2], op=Alu.subtract)
            nc.vector.tensor_tensor(Si[:, 14:15], Si[:, 4:5], Si[:, 2:3], op=Alu.subtract)
            scal2f = pool.tile([1, 2], f32)
            scal2s = pool.tile([1, 2], f32)
            nc.vector.tensor_copy(scal2f[:, 0:1], Si[:, 12:13])
            nc.vector.tensor_copy(scal2f[:, 1:2], Si[:, 14:15])
            nc.vector.tensor_copy(scal2s[:, 0:1], Si[:, 13:14])
            nc.vector.tensor_copy(scal2s[:, 1:2], Si[:, 14:15])

            # broadcast deltas across partitions via K=1 matmuls; PSUM is
            # staged through SBUF so mask math carries DVE-only waits.
            # bc_sb columns: 0 dRf, 1 dC, 2 dRs, 3 dC, 4 r2
            bcf = psum.tile([128, 2], f32)
            bcs = psum.tile([128, 2], f32)
            nc.tensor.matmul(bcf, ones, scal2f, start=True, stop=True)
            nc.tensor.matmul(bcs, ones, scal2s, start=True, stop=True)
            bc_sb = pool.tile([128, 5], f32)
            nc.vector.tensor_copy(bc_sb[:, 0:2], bcf[:, :])
            nc.vector.tensor_copy(bc_sb[:, 2:4], bcs[:, :])

            # rad-independent mask halves: sum2 = (dR+p)^2 + (dC+c)^2
            dr = pool.tile([128, 1], f32)
            dr2 = pool.tile([128, 1], f32)
            dc = pool.tile([128, CROP], f32)
            sum2F = pool.tile([128, CROP], f32)
            sum2S = pool.tile([128, CROP], f32)

            def emit_sum2(drcol, dccol, sum2):
                nc.vector.tensor_tensor(dr, iota_p, drcol, op=Alu.add)
                nc.vector.tensor_tensor(dr2, dr, dr, op=Alu.mult)
                nc.vector.tensor_scalar(dc, iota_f, dccol, None, op0=Alu.add)
                nc.vector.tensor_tensor(sum2, dc, dc, op=Alu.mult)
                nc.vector.tensor_tensor(
                    sum2, sum2, dr2.to_broadcast([128, CROP]), op=Alu.add
                )

            emit_sum2(bc_sb[:, 0:1], bc_sb[:, 1:2], sum2F)
            emit_sum2(bc_sb[:, 2:3], bc_sb[:, 3:4], sum2S)

            # ---------- rad = clip(floor(r_val + 8), 0, 254) ----------
            nc.vector.tensor_scalar(Sf[:, 1:2], rv, float(R_EXPAND), None, op0=Alu.add)
            nc.vector.tensor_copy(Si[:, 9:10], Sf[:, 1:2])  # c1 (unknown rounding)
            nc.vector.tensor_copy(Sf[:, 2:3], Si[:, 9:10])  # c1 back to f32
            nc.vector.tensor_tensor(
                Sf[:, 3:4], Sf[:, 2:3], Sf[:, 1:2], op=Alu.is_gt
            )  # c1 > x -> overshoot by 1
            nc.vector.tensor_copy(Si[:, 10:11], Sf[:, 3:4])
            nc.vector.tensor_tensor(
                Si[:, 11:12], Si[:, 9:10], Si[:, 10:11], op=Alu.subtract
            )  # floor
            nc.vector.tensor_scalar(
                Si[:, 11:12], Si[:, 11:12], 0, R_MAX - 1, op0=Alu.max, op1=Alu.min
            )
            nc.vector.tensor_copy(Sf[:, 4:5], Si[:, 11:12])  # rad as f32
            r2sb = pool.tile([1, 1], f32)
            nc.vector.tensor_tensor(r2sb, Sf[:, 4:5], Sf[:, 4:5], op=Alu.mult)
            bc2 = psum.tile([128, 1], f32)
            nc.tensor.matmul(bc2, ones, r2sb, start=True, stop=True)
            nc.vector.tensor_copy(bc_sb[:, 4:5], bc2[:, :])

            radv = nc.values_load(
                Si[:, 11:12],
                engines=RADV_ENGINES,
                min_val=0,
                max_val=R_MAX - 1,
                skip_runtime_bounds_check=True,
            )

            # final masks (1.0 inside the disc)
            maskF = pool.tile([128, CROP], f32)
            maskS = pool.tile([128, CROP], f32)
            nc.vector.tensor_scalar(maskF, sum2F, bc_sb[:, 4:5], None, op0=Alu.is_le)
            nc.vector.tensor_scalar(maskS, sum2S, bc_sb[:, 4:5], None, op0=Alu.is_le)

            # fast-path mask replicated to all 4 partition groups
            mask4 = pool.tile([128, CROP], f32)
            nc.vector.tensor_copy(mask4[0:32, :], maskF[0:32, :])
            nc.vector.tensor_copy(mask4[32:64, :], maskF[0:32, :])
            nc.vector.tensor_copy(mask4[64:96, :], maskF[0:32, :])
            nc.vector.tensor_copy(mask4[96:128, :], maskF[0:32, :])

            # ---------- per-path tiles (disjoint across the If arms) ----------
            # fast: [128, CG*CROP]; partition 32g+r = window row r, channels
            #       [CG*g, CG*(g+1)); free = (c_lo, col)
            crop_tF = pool.tile([128, CG * CROP], f32)
            out_tF = pool.tile([128, CG * CROP], f32)
            junkF = pool.tile([128, 1], f32)
            junkP = pool.tile([128, 1], f32)
            junkD = pool.tile([1, 4], f32)
            # slow: [128, C*CROP]; partition = crop row; free = (chan, col)
            crop_tS = pool.tile([128, C * CROP], f32)
            out_tS = pool.tile([128, C * CROP], f32)
            junkS = pool.tile([128, 1], f32)

            fm_rows = fm.transpose([1, 0, 2])  # [H, C, W]
            out_rows = out.transpose([1, 0, 2])  # [CROP, C, CROP]

            def q(g):
                # alternate the two HWDGE rings so descriptor generation for
                # the 4 group-DMAs runs on two sequencers in parallel
                return nc.sync if g % 2 == 0 else nc.scalar

            # prefetch the fast-path window crop unconditionally (harmless on
            # the slow path); overlaps the rad chain / mask math above
            for g in range(4):
                ps = slice(32 * g, 32 * (g + 1))
                q(g).dma_start(
                    crop_tF[ps, :],
                    fm_rows[ds(grv, WIN), CG * g : CG * (g + 1), ds(leftv, CROP)],
                )
                # bridge: absorb each DMA lane's wait outside the If
                nc.vector.tensor_copy(
                    junkF[32 * g : 32 * g + 1, :],
                    crop_tF[32 * g : 32 * g + 1, 0:1],
                )

            def fast_body():
                # pads keep the arm's DVE count equal to the slow arm's
                pads = [
                    nc.vector.tensor_copy(
                        junkP[32 * g : 32 * g + 1, :],
                        junkF[32 * g : 32 * g + 1, :],
                    )
                    for g in range(4)
                ]
                mul = nc.vector.tensor_tensor(
                    out_tF[:, :],
                    crop_tF[:, :],
                    mask4[:, :]
                    .rearrange("p (o w) -> p o w", o=1)
                    .to_broadcast([128, CG, CROP]),
                    op=Alu.mult,
                )
                for g in range(4):
                    ps = slice(32 * g, 32 * (g + 1))
                    q(g).dma_start(
                        out_rows[ds(wrv, WIN), CG * g : CG * (g + 1), :],
                        out_tF[ps, :],
                    )
                # dummy 4B loads keep the arm's DMA count equal (lane ticks)
                for g in range(4):
                    q(g).dma_start(junkD[:, g : g + 1], wh[0:1, 0:1])

            def slow_body():
                # full 128x128 crop, 4 channel-group DMAs each way
                bridges = []
                for g in range(4):
                    fs = slice(CG * CROP * g, CG * CROP * (g + 1))
                    q(g).dma_start(
                        crop_tS[:, fs],
                        fm_rows[ds(topv, CROP), CG * g : CG * (g + 1), ds(leftv, CROP)],
                    )
                    bridges.append(
                        nc.vector.tensor_copy(
                            junkS[0:1, :], crop_tS[0:1, CG * CROP * g : CG * CROP * g + 1]
                        )
                    )
                mul = nc.vector.tensor_tensor(
                    out_tS[:, :],
                    crop_tS[:, :],
                    maskS[:, :]
                    .rearrange("p (o w) -> p o w", o=1)
                    .to_broadcast([128, C, CROP]),
                    op=Alu.mult,
                )
                for br in bridges:
                    add_dep_helper(mul.ins, br.ins, sync=False, reason="wait bridge")
                for g in range(4):
                    fs = slice(CG * CROP * g, CG * CROP * (g + 1))
                    q(g).dma_start(
                        out_rows[:, CG * g : CG * (g + 1), :], out_tS[:, fs]
                    )

            if use_if:
                with tc.If(radv < 16) as cmp:
                    fast_body()
                with cmp.Else():
                    slow_body()
            else:
                slow_body()

    nc.compile()
    return nc


_NC_CACHE = None


def _get_nc() -> bass.Bass:
    global _NC_CACHE
    if _NC_CACHE is None:
        _NC_CACHE = build_nc()
    return _NC_CACHE


def make_in_maps(feature_map, heatmap, outer_wh):
    cst = _consts_np()
    in_maps = []
    for i in range(N_CORES):
        in_maps.append(
            {
                "heat": np.ascontiguousarray(heatmap[i, 1]).astype(np.float32),
                "wh": np.ascontiguousarray(outer_wh[i, 0]).astype(np.float32),
                "fm": np.ascontiguousarray(feature_map[i]).astype(np.float32),
                "consts": cst,
            }
        )
    return in_maps


def run(feature_map, heatmap, outer_wh, trace=False):
    from concourse.bass_utils import run_bass_kernel_spmd

    nc = _get_nc()
    in_maps = make_in_maps(feature_map, heatmap, outer_wh)
    res = run_bass_kernel_spmd(nc, in_maps, list(range(N_CORES)), trace=trace)
    focus = np.stack([np.asarray(res.results[i]["out"]) for i in range(N_CORES)])
    hook = np.stack(
        [np.asarray(res.results[i]["hook"]).astype(np.int32) for i in range(N_CORES)]
    )
    return (focus, hook), res


def kernel(feature_map, heatmap, outer_wh):
    (focus, hook), _ = run(feature_map, heatmap, outer_wh, trace=False)
    return focus, hook
